# revision 6
# baseline (speedup 1.0000x reference)
"""Bhattacharyya coefficient kernel for Trainium2 (8 NeuronCores, SPMD).

out[n,0,i,j] = (1/k^2) * sum_{c,p,q} w[c] * sqrt(x[n,c,i+p,j+q] * z[n,c,p,q])

Data-parallel over batch: 2 samples per core. Per sample:
  1. ACT: sx = sqrt(x) (bf16), szw = w/k^2 * sqrt(z) (bf16).
  2. TensorE: plane[t, y] = sum_c szw[c, t] * sx[c, y] for the 64 taps
     t = 8p+q and all 63*63 image pixels y (K=256 in two 128-chunks
     accumulated in PSUM, M=64 taps, N in blocks of <=512).
  3. Evict PSUM -> SBUF (bf16, DVE), dump plane to DRAM scratch.
  4. Gather back with per-tap shifted offsets (flat DRAM AP):
     A[t, 441*r + u] = plane[t, 441*r + u + 63*(t>>3) + (t&7)] for 8
     out-row chunks r (7 output rows each), turning the tap-sum into a
     pure partition reduction.
  5. TensorE one-hot matmuls: ps[r, u] = sum_t A[t, 441*r + u] via a
     [64, 8] stationary whose only nonzero column is r, accumulating all
     8 chunks into one [8, 441] PSUM tile.  One DVE eviction compacts
     the valid 7x56 block per partition into obuf [8, 392]; one DMA
     ships the contiguous [56, 56] output.

Sample 0's x is loaded with a single 4MB DMA; sample 1 is split into
2048/1536/385-column pieces per half of the channels so the serial
chain behind the last DMA byte stays short.  z/w ride the Scalar HWDGE
ring so they land early without blocking the x stream on Sync.
"""

import numpy as np

import concourse.bacc as bacc
import concourse.bass as bass
import concourse.mybir as mybir
from concourse import tile
from concourse.bass_utils import run_bass_kernel_spmd

N, C, KS, MS = 16, 256, 8, 63
MO = MS - KS + 1            # 56
F = MS * MS                 # 3969
NCORES = 8
SPC = N // NCORES           # samples per core
BLK = 512
AF = mybir.ActivationFunctionType
f32 = mybir.dt.float32
bf16 = mybir.dt.bfloat16

RT = bf16                   # round-trip dtype for plane scratch

# x staging pieces for sample 1 (start block, n blocks): 4 + 3 + 1
PIECES = [(0, 4), (4, 3), (7, 1)]
NBLOCKS = [(min(BLK, F - b * BLK)) for b in range(8)]   # 512 x7, 385

# stage-2: 8 out-row chunks of 7 rows; chunk r covers flat u in
# [441r, 441r+441), gathers plane cols [441r, 441r+889).
CW = 441
# scratch A: plane cols [0, 3072) (blocks 0-5)  -> chunks 0-4
# scratch B: plane cols [2205, 3976)            -> chunks 5-7
PIT_A = 6 * BLK             # 3072
B_LO = 5 * CW               # 2205
PIT_B = 1771                # 2205 + 1771 = 3976

_CACHE = {}


def _build():
    nc = bacc.Bacc("TRN2", target_bir_lowering=False, debug=False)
    z_in = nc.declare_dram_parameter("z", [SPC, C, KS, KS], f32, isOutput=False)
    x_in = nc.declare_dram_parameter("x", [SPC, C, MS, MS], f32, isOutput=False)
    w_in = nc.declare_dram_parameter("w", [C], f32, isOutput=False)
    out = nc.declare_dram_parameter("out", [SPC, 1, MO, MO], f32, isOutput=True)

    scA = [nc.dram_tensor(f"pl_scA{s}", [64, PIT_A], RT) for s in range(SPC)]
    scB = [nc.dram_tensor(f"pl_scB{s}", [64, PIT_B], RT) for s in range(SPC)]

    xflat = x_in.rearrange("s (k c) h w -> s k c (h w)", c=128)  # [SPC,2,128,F]

    with tile.TileContext(nc) as tc:
        with (
            tc.tile_pool(name="xs0p", bufs=1) as xs0p,
            tc.tile_pool(name="xstage", bufs=6) as xstage,
            tc.tile_pool(name="sx0p", bufs=1) as sx0p,
            tc.tile_pool(name="sxq", bufs=6) as sxq,
            tc.tile_pool(name="zpool", bufs=8) as zpool,
            tc.tile_pool(name="plApool", bufs=2) as plApool,
            tc.tile_pool(name="pl7pool", bufs=2) as pl7pool,
            tc.tile_pool(name="g1pool", bufs=2) as g1pool,
            tc.tile_pool(name="g2pool", bufs=2) as g2pool,
            tc.tile_pool(name="ohpool", bufs=8) as ohpool,
            tc.tile_pool(name="obpool", bufs=2) as obpool,
            tc.tile_pool(name="psum", bufs=4, space="PSUM") as psum,
            tc.tile_pool(name="psum2", bufs=2, space="PSUM") as psum2,
        ):
            # one-hot [64, 8] stationaries for the chunked tap reduction
            ohs = []
            for r in range(8):
                oh = ohpool.tile([64, 8], RT, name=f"oh{r}")
                nc.gpsimd.memset(oh[:], 0.0)
                nc.gpsimd.memset(oh[:, r : r + 1], 1.0)
                ohs.append(oh)

            # ---- input DMAs ----
            # z/w on the Scalar HWDGE ring: early, off the x stream
            wt = zpool.tile([128, 2], f32, name="wt")
            nc.scalar.dma_start(wt[:], w_in.rearrange("(k c) -> c k", c=128))
            zts = []
            for s in range(SPC):
                zt = zpool.tile([128, 2, KS * KS], f32, tag="zt", name=f"zt{s}")
                nc.scalar.dma_start(
                    zt[:], z_in[s].rearrange("(k c) p q -> c k (p q)", c=128)
                )
                zts.append(zt)
            w64 = zpool.tile([128, 2], f32, name="w64")
            nc.vector.tensor_scalar_mul(w64[:], wt[:], 1.0 / (KS * KS))

            # x on the Sync HWDGE ring, in consumption order:
            # sample 0 in one 4MB load, sample 1 in 6 pieces.
            xs0 = xs0p.tile([128, 2, F], f32, name="xs0")
            nc.sync.dma_start(
                xs0[:], x_in[0].rearrange("(k c) h w -> c k (h w)", c=128)
            )
            xst1 = {}
            for pi, (b0, nbk) in enumerate(PIECES):
                for k in range(2):
                    lo = b0 * BLK
                    ln = min(nbk * BLK, F - lo)
                    t = xstage.tile([128, 4 * BLK], f32, tag="xst",
                                    name=f"x1_{k}{pi}")
                    nc.sync.dma_start(t[:, :ln], xflat[1, k, :, lo : lo + ln])
                    xst1[(k, pi)] = t

            # ---- z path: szw[c, k, t] = w[c]/64 * sqrt(z[c, t]) ----
            szws = []
            for s in range(SPC):
                zsq = zpool.tile([128, 2, KS * KS], f32, tag="zsq", name=f"zsq{s}")
                szw = zpool.tile([128, 2, KS * KS], bf16, tag="szw", name=f"szw{s}")
                for k in range(2):
                    nc.scalar.activation(zsq[:, k, :], zts[s][:, k, :], AF.Sqrt)
                    nc.vector.tensor_scalar_mul(
                        szw[:, k, :], zsq[:, k, :], w64[:, k : k + 1]
                    )
                szws.append(szw)

            # ---- stage 1 ----
            def stage1(s, sx_of_block):
                """sx_of_block(k, b) -> (tile_ap, col_offset_of_block_in_tile)"""
                szw = szws[s]
                plA = plApool.tile([64, 7 * BLK], RT, tag="plA", name=f"plA{s}")
                pl7 = pl7pool.tile([64, 385], RT, tag="pl7", name=f"pl7{s}")
                for b in range(8):
                    nb = NBLOCKS[b]
                    ps = psum.tile([64, BLK], f32, tag="ps", name=f"ps{s}_{b}")
                    for k in range(2):
                        src, off = sx_of_block(k, b)
                        nc.tensor.matmul(
                            ps[:, :nb],
                            szw[:, k, :],
                            src[:, off : off + nb],
                            start=(k == 0),
                            stop=(k == 1),
                        )
                    if b < 7:
                        nc.vector.tensor_copy(
                            plA[:, b * BLK : b * BLK + nb], ps[:, :nb]
                        )
                    else:
                        nc.vector.tensor_copy(pl7[:, :nb], ps[:, :nb])
                    # dumps as soon as their source region is complete
                    if b == 5:
                        nc.sync.dma_start(scA[s][:, :PIT_A], plA[:, :PIT_A])
                    elif b == 6:
                        nc.sync.dma_start(
                            scB[s][:, 0 : 3584 - B_LO], plA[:, B_LO:3584]
                        )
                    elif b == 7:
                        nc.sync.dma_start(
                            scB[s][:, 3584 - B_LO : 3584 - B_LO + 385], pl7[:, :385]
                        )

            # sample 0: sqrt in two [128, F] halves from the merged tile
            sx0 = sx0p.tile([128, 2, F], bf16, name="sx0")
            for k in range(2):
                nc.scalar.activation(sx0[:, k, :], xs0[:, k, :], AF.Sqrt)
            stage1(0, lambda k, b: (sx0[:, k, :], b * BLK))

            # sample 1: per-piece sqrt
            sx1 = {}
            for pi, (b0, nbk) in enumerate(PIECES):
                lo = b0 * BLK
                ln = min(nbk * BLK, F - lo)
                for k in range(2):
                    t = sxq.tile([128, 4 * BLK], bf16, tag="sxp",
                                 name=f"sx1_{k}{pi}")
                    nc.scalar.activation(
                        t[:, :ln], xst1[(k, pi)][:, :ln], AF.Sqrt
                    )
                    sx1[(k, pi)] = t

            def sx1_of_block(k, b):
                pi = 0 if b < 4 else (1 if b < 7 else 2)
                b0 = PIECES[pi][0]
                return sx1[(k, pi)], (b - b0) * BLK

            stage1(1, sx1_of_block)

            # ---- stage 2 ----
            for s in range(SPC):
                g1 = g1pool.tile([64, 5 * CW], RT, tag="g1", name=f"g1_{s}")
                srcA = bass.AP(
                    scA[s][:].tensor, 0,
                    [[8 * PIT_A + MS, 8], [PIT_A + 1, 8], [CW, 5], [1, CW]],
                )
                g2 = g2pool.tile([64, 3 * CW], RT, tag="g2", name=f"g2_{s}")
                srcB = bass.AP(
                    scB[s][:].tensor, 0,
                    [[8 * PIT_B + MS, 8], [PIT_B + 1, 8], [CW, 3], [1, CW]],
                )
                if s == 0:
                    nc.gpsimd.dma_start(g1[:], srcA)
                    nc.gpsimd.dma_start(g2[:], srcB)
                else:
                    nc.gpsimd.dma_start(g1[:], srcA)
                    nc.sync.dma_start(g2[:], srcB)

                ps2 = psum2.tile([8, CW], f32, tag="ps2", name=f"ps2_{s}")
                for r in range(8):
                    src = g1 if r < 5 else g2
                    off = (r if r < 5 else r - 5) * CW
                    nc.tensor.matmul(
                        ps2[:, :CW],
                        ohs[r][:],
                        src[:, off : off + CW],
                        start=(r == 0),
                        stop=(r == 7),
                    )

                obuf = obpool.tile([8, 7 * MO], f32, tag="ob", name=f"ob{s}")
                psv = ps2[:].rearrange("p (i j) -> p i j", j=MS)[:, :, 0:MO]
                nc.vector.tensor_copy(
                    obuf[:].rearrange("p (i j) -> p i j", j=MO), psv
                )
                nc.scalar.dma_start(
                    out[s].rearrange("c (r i) j -> (c r) (i j)", r=8), obuf[:]
                )

    nc.compile()
    return nc


def _get_nc():
    if "nc" not in _CACHE:
        _CACHE["nc"] = _build()
    return _CACHE["nc"]


def _run(z, x, weights, **runkw):
    z = np.ascontiguousarray(np.asarray(z), dtype=np.float32)
    x = np.ascontiguousarray(np.asarray(x), dtype=np.float32)
    w = np.ascontiguousarray(np.asarray(weights), dtype=np.float32).reshape(C)
    in_maps = []
    for i in range(NCORES):
        lo, hi = i * SPC, (i + 1) * SPC
        in_maps.append({"z": z[lo:hi], "x": x[lo:hi], "w": w})
    nc = _get_nc()
    try:
        res = run_bass_kernel_spmd(
            nc, in_maps, core_ids=list(range(NCORES)), **runkw
        )
    except Exception:
        # transient device errors (e.g. NRT exec-unit unrecoverable) have
        # been observed to succeed on retry
        res = run_bass_kernel_spmd(
            nc, in_maps, core_ids=list(range(NCORES)), **runkw
        )
    full = np.concatenate([res.results[i]["out"] for i in range(NCORES)], axis=0)
    return full, res


def kernel(z, x, weights):
    full, _ = _run(z, x, weights)
    return full


# revision 8
# speedup vs baseline: 1.2168x; 1.2168x over previous
"""Bhattacharyya coefficient kernel for Trainium2 (8 NeuronCores, SPMD).

out[n,0,i,j] = (1/k^2) * sum_{c,p,q} w[c] * sqrt(x[n,c,i+p,j+q] * z[n,c,p,q])

Data-parallel over batch: 2 samples per core. Per sample:
  1. ACT: sx = sqrt(x) (bf16), szw = w/k^2 * sqrt(z) (bf16).
  2. TensorE: plane[t, y] = sum_c szw[c, t] * sx[c, y] for the 64 taps
     t = 8p+q and all 63*63 image pixels y (K=256 in two 128-chunks
     accumulated in PSUM, M=64 taps, N in blocks of <=512).
  3. Evict PSUM -> SBUF (bf16, DVE), dump plane to DRAM scratch.
  4. Gather back with per-tap shifted offsets (flat DRAM AP):
     A[t, 441*r + u] = plane[t, 441*r + u + 63*(t>>3) + (t&7)] for 8
     out-row chunks r (7 output rows each), turning the tap-sum into a
     pure partition reduction.  Three scratch tensors (chunks 0-4 /
     5-6 / 7) keep each gather waiting only on the dumps it covers;
     the chunk-7 gather -- the only one behind the last x piece -- is
     a single small HWDGE transfer.
  5. TensorE one-hot matmuls: ps[r, u] = sum_t A[t, 441*r + u] via a
     [64, 8] stationary whose only nonzero column is r, accumulating all
     8 chunks into one [8, 441] PSUM tile.  One DVE eviction compacts
     the valid 7x56 block per partition into obuf [8, 392]; one DMA
     ships the contiguous [56, 56] output.

x is loaded in 2048/1536/385-column pieces covering both channel
halves at once ([128, 2, cols] tiles); z/w ride the GpSimd SWDGE ring
(issued first) so they land early without queueing behind the x stream.
"""

import numpy as np

import concourse.bacc as bacc
import concourse.bass as bass
import concourse.mybir as mybir
from concourse import tile
from concourse.bass_utils import run_bass_kernel_spmd

N, C, KS, MS = 16, 256, 8, 63
MO = MS - KS + 1            # 56
F = MS * MS                 # 3969
NCORES = 8
SPC = N // NCORES           # samples per core
BLK = 512
AF = mybir.ActivationFunctionType
f32 = mybir.dt.float32
bf16 = mybir.dt.bfloat16

RT = bf16                   # round-trip dtype for plane scratch

# x staging pieces (start block, n blocks): 4 + 3 + 1, both k halves each
PIECES = [(0, 4), (4, 3), (7, 1)]
NBLOCKS = [(min(BLK, F - b * BLK)) for b in range(8)]   # 512 x7, 385

# stage-2: 8 out-row chunks of 7 rows; chunk r covers flat u in
# [441r, 441r+441), gathers plane cols [441r, 441r+889).
CW = 441
PIT_A = 6 * BLK             # scA: plane cols [0, 3072)      -> chunks 0-4
B_LO = 5 * CW               # 2205
PIT_B = 3584 - B_LO         # scB: plane cols [2205, 3584)   -> chunks 5-6
C_LO = 7 * CW               # 3087
PIT_C = 889                 # scC: plane cols [3087, 3976)   -> chunk 7

_CACHE = {}


def _build():
    nc = bacc.Bacc("TRN2", target_bir_lowering=False, debug=False)
    z_in = nc.declare_dram_parameter("z", [SPC, C, KS, KS], f32, isOutput=False)
    x_in = nc.declare_dram_parameter("x", [SPC, C, MS, MS], f32, isOutput=False)
    w_in = nc.declare_dram_parameter("w", [C], f32, isOutput=False)
    out = nc.declare_dram_parameter("out", [SPC, 1, MO, MO], f32, isOutput=True)

    scA = [nc.dram_tensor(f"pl_scA{s}", [64, PIT_A], RT) for s in range(SPC)]
    scB = [nc.dram_tensor(f"pl_scB{s}", [64, PIT_B], RT) for s in range(SPC)]
    scC = [nc.dram_tensor(f"pl_scC{s}", [64, PIT_C], RT) for s in range(SPC)]

    # [SPC, 128, 2, F]: partition = channel-within-half, then half, pixels
    xsrc = x_in.rearrange("s (k c) h w -> s c k (h w)", c=128)

    with tile.TileContext(nc) as tc:
        with (
            tc.tile_pool(name="xstage", bufs=6) as xstage,
            tc.tile_pool(name="sxq", bufs=6) as sxq,
            tc.tile_pool(name="zpool", bufs=8) as zpool,
            tc.tile_pool(name="plApool", bufs=2) as plApool,
            tc.tile_pool(name="pl7pool", bufs=2) as pl7pool,
            tc.tile_pool(name="g1pool", bufs=2) as g1pool,
            tc.tile_pool(name="g2pool", bufs=2) as g2pool,
            tc.tile_pool(name="g3pool", bufs=2) as g3pool,
            tc.tile_pool(name="ohpool", bufs=8) as ohpool,
            tc.tile_pool(name="obpool", bufs=2) as obpool,
            tc.tile_pool(name="psum", bufs=4, space="PSUM") as psum,
            tc.tile_pool(name="psum2", bufs=2, space="PSUM") as psum2,
        ):
            # z/w first on the SWDGE ring so they land early
            wt = zpool.tile([128, 2], f32, name="wt")
            nc.gpsimd.dma_start(wt[:], w_in.rearrange("(k c) -> c k", c=128))
            zts = []
            for s in range(SPC):
                zt = zpool.tile([128, 2, KS * KS], f32, tag="zt", name=f"zt{s}")
                nc.gpsimd.dma_start(
                    zt[:], z_in[s].rearrange("(k c) p q -> c k (p q)", c=128)
                )
                zts.append(zt)
            w64 = zpool.tile([128, 2], f32, name="w64")
            nc.vector.tensor_scalar_mul(w64[:], wt[:], 1.0 / (KS * KS))

            # one-hot [64, 8] stationaries for the chunked tap reduction
            ohs = []
            for r in range(8):
                oh = ohpool.tile([64, 8], RT, name=f"oh{r}")
                nc.gpsimd.memset(oh[:], 0.0)
                nc.gpsimd.memset(oh[:, r : r + 1], 1.0)
                ohs.append(oh)

            # x pieces on the Sync HWDGE ring, in consumption order
            xst = {}
            for s in range(SPC):
                for pi, (b0, nbk) in enumerate(PIECES):
                    lo = b0 * BLK
                    ln = min(nbk * BLK, F - lo)
                    t = xstage.tile([128, 2, 4 * BLK], f32, tag="xst",
                                    name=f"x{s}p{pi}")
                    nc.sync.dma_start(t[:, :, :ln], xsrc[s, :, :, lo : lo + ln])
                    xst[(s, pi)] = t

            # ---- z path: szw[c, k, t] = w[c]/64 * sqrt(z[c, t]) ----
            szws = []
            for s in range(SPC):
                zsq = zpool.tile([128, 2, KS * KS], f32, tag="zsq", name=f"zsq{s}")
                szw = zpool.tile([128, 2, KS * KS], bf16, tag="szw", name=f"szw{s}")
                for k in range(2):
                    nc.scalar.activation(zsq[:, k, :], zts[s][:, k, :], AF.Sqrt)
                    nc.vector.tensor_scalar_mul(
                        szw[:, k, :], zsq[:, k, :], w64[:, k : k + 1]
                    )
                szws.append(szw)

            # ---- stage 1 ----
            sxt = {}

            def sqrt_piece(s, pi):
                b0, nbk = PIECES[pi]
                ln = min(nbk * BLK, F - b0 * BLK)
                t = sxq.tile([128, 2, 4 * BLK], bf16, tag="sxp",
                             name=f"sx{s}p{pi}")
                nc.scalar.activation(
                    t[:, :, :ln], xst[(s, pi)][:, :, :ln], AF.Sqrt
                )
                sxt[(s, pi)] = t

            def stage1(s):
                szw = szws[s]
                plA = plApool.tile([64, 7 * BLK], RT, tag="plA", name=f"plA{s}")
                pl7 = pl7pool.tile([64, 385], RT, tag="pl7", name=f"pl7{s}")
                for b in range(8):
                    nb = NBLOCKS[b]
                    pi = 0 if b < 4 else (1 if b < 7 else 2)
                    off = (b - PIECES[pi][0]) * BLK
                    sx = sxt[(s, pi)]
                    ps = psum.tile([64, BLK], f32, tag="ps", name=f"ps{s}_{b}")
                    for k in range(2):
                        nc.tensor.matmul(
                            ps[:, :nb],
                            szw[:, k, :],
                            sx[:, k, off : off + nb],
                            start=(k == 0),
                            stop=(k == 1),
                        )
                    if b < 7:
                        nc.vector.tensor_copy(
                            plA[:, b * BLK : b * BLK + nb], ps[:, :nb]
                        )
                    else:
                        nc.vector.tensor_copy(pl7[:, :nb], ps[:, :nb])
                    # dumps as soon as their source region is complete
                    if b == 5:
                        nc.sync.dma_start(scA[s][:, :PIT_A], plA[:, :PIT_A])
                    elif b == 6:
                        nc.sync.dma_start(scB[s][:], plA[:, B_LO:3584])
                        nc.sync.dma_start(
                            scC[s][:, 0 : 3584 - C_LO], plA[:, C_LO:3584]
                        )
                    elif b == 7:
                        nc.sync.dma_start(
                            scC[s][:, 3584 - C_LO : 3584 - C_LO + 385],
                            pl7[:, :385],
                        )

            for s in range(SPC):
                for pi in range(3):
                    sqrt_piece(s, pi)
                stage1(s)

            # ---- stage 2 ----
            def gathers(s):
                g1 = g1pool.tile([64, 5 * CW], RT, tag="g1", name=f"g1_{s}")
                nc.gpsimd.dma_start(g1[:], bass.AP(
                    scA[s][:].tensor, 0,
                    [[8 * PIT_A + MS, 8], [PIT_A + 1, 8], [CW, 5], [1, CW]],
                ))
                g2 = g2pool.tile([64, 2 * CW], RT, tag="g2", name=f"g2_{s}")
                nc.gpsimd.dma_start(g2[:], bass.AP(
                    scB[s][:].tensor, 0,
                    [[8 * PIT_B + MS, 8], [PIT_B + 1, 8], [CW, 2], [1, CW]],
                ))
                g3 = g3pool.tile([64, CW], RT, tag="g3", name=f"g3_{s}")
                src3 = bass.AP(
                    scC[s][:].tensor, 0,
                    [[8 * PIT_C + MS, 8], [PIT_C + 1, 8], [1, CW]],
                )
                if s == 0:
                    nc.gpsimd.dma_start(g3[:], src3)
                else:
                    nc.sync.dma_start(g3[:], src3)
                return g1, g2, g3

            gts = [gathers(s) for s in range(SPC)]

            for s in range(SPC):
                g1, g2, g3 = gts[s]
                ps2 = psum2.tile([8, CW], f32, tag="ps2", name=f"ps2_{s}")
                for r in range(8):
                    src = g1 if r < 5 else (g2 if r < 7 else g3)
                    off = (r if r < 5 else (r - 5 if r < 7 else 0)) * CW
                    nc.tensor.matmul(
                        ps2[:, :CW],
                        ohs[r][:],
                        src[:, off : off + CW],
                        start=(r == 0),
                        stop=(r == 7),
                    )

                obuf = obpool.tile([8, 7 * MO], f32, tag="ob", name=f"ob{s}")
                psv = ps2[:].rearrange("p (i j) -> p i j", j=MS)[:, :, 0:MO]
                nc.vector.tensor_copy(
                    obuf[:].rearrange("p (i j) -> p i j", j=MO), psv
                )
                nc.scalar.dma_start(
                    out[s].rearrange("c (r i) j -> (c r) (i j)", r=8), obuf[:]
                )

    nc.compile()
    return nc


def _get_nc():
    if "nc" not in _CACHE:
        _CACHE["nc"] = _build()
    return _CACHE["nc"]


def _run(z, x, weights, **runkw):
    z = np.ascontiguousarray(np.asarray(z), dtype=np.float32)
    x = np.ascontiguousarray(np.asarray(x), dtype=np.float32)
    w = np.ascontiguousarray(np.asarray(weights), dtype=np.float32).reshape(C)
    in_maps = []
    for i in range(NCORES):
        lo, hi = i * SPC, (i + 1) * SPC
        in_maps.append({"z": z[lo:hi], "x": x[lo:hi], "w": w})
    nc = _get_nc()
    try:
        res = run_bass_kernel_spmd(
            nc, in_maps, core_ids=list(range(NCORES)), **runkw
        )
    except Exception:
        # transient device errors (e.g. NRT exec-unit unrecoverable) have
        # been observed to succeed on retry
        res = run_bass_kernel_spmd(
            nc, in_maps, core_ids=list(range(NCORES)), **runkw
        )
    full = np.concatenate([res.results[i]["out"] for i in range(NCORES)], axis=0)
    return full, res


def kernel(z, x, weights):
    full, _ = _run(z, x, weights)
    return full


# revision 10
# speedup vs baseline: 1.2243x; 1.0061x over previous
"""Bhattacharyya coefficient kernel for Trainium2 (8 NeuronCores, SPMD).

out[n,0,i,j] = (1/k^2) * sum_{c,p,q} w[c] * sqrt(x[n,c,i+p,j+q] * z[n,c,p,q])

Data-parallel over batch: 2 samples per core. Per sample:
  1. ACT: sx = sqrt(x) (bf16), szw = w/k^2 * sqrt(z) (bf16).
  2. TensorE: plane[t, y] = sum_c szw[c, t] * sx[c, y] for the 64 taps
     t = 8p+q and all 63*63 image pixels y (K=256 in two 128-chunks
     accumulated in PSUM, M=64 taps, N in blocks of <=512).
  3. Evict PSUM -> SBUF (bf16, DVE), dump plane to DRAM scratch.
  4. Gather back with per-tap shifted offsets (flat DRAM AP):
     A[t, 441*r + u] = plane[t, 441*r + u + 63*(t>>3) + (t&7)] for 8
     out-row chunks r (7 output rows each), turning the tap-sum into a
     pure partition reduction.  Three scratch tensors (chunks 0-4 /
     5-6 / 7) keep each gather waiting only on the dumps it covers.
  5. TensorE one-hot matmuls: ps[r, u] = sum_t A[t, 441*r + u] via a
     [64, 8] stationary whose only nonzero column is r, accumulating all
     8 chunks into one [8, 441] PSUM tile.  One DVE eviction compacts
     the valid 7x56 block per partition into obuf [8, 392]; one DMA
     ships the contiguous [56, 56] output.

x arrives in decreasing-size pieces (2048/1024/512/385 cols, both
channel halves per piece) with the two samples' pieces interleaved, so
both samples' serial tails (last piece -> sqrt -> matmul -> dump ->
gather -> reduce -> out) run concurrently right after the stream ends.
All gathers ride HWDGE rings (SWDGE completion is ~6us); z/w ride
SWDGE early where they don't contend with the x stream.
"""

import numpy as np

import concourse.bacc as bacc
import concourse.bass as bass
import concourse.mybir as mybir
from concourse import tile
from concourse.bass_utils import run_bass_kernel_spmd

N, C, KS, MS = 16, 256, 8, 63
MO = MS - KS + 1            # 56
F = MS * MS                 # 3969
NCORES = 8
SPC = N // NCORES           # samples per core
BLK = 512
AF = mybir.ActivationFunctionType
f32 = mybir.dt.float32
bf16 = mybir.dt.bfloat16

RT = bf16                   # round-trip dtype for plane scratch

# x pieces (start block, n blocks): decreasing sizes
PIECES = [(0, 4), (4, 2), (6, 1), (7, 1)]
NBLOCKS = [(min(BLK, F - b * BLK)) for b in range(8)]   # 512 x7, 385

# stage-2: 8 out-row chunks of 7 rows; chunk r covers flat u in
# [441r, 441r+441), gathers plane cols [441r, 441r+889).
CW = 441
PIT_A = 6 * BLK             # scA: plane cols [0, 3072)      -> chunks 0-4
B_LO = 5 * CW               # 2205
PIT_B = 3584 - B_LO         # scB: plane cols [2205, 3584)   -> chunks 5-6
C_LO = 7 * CW               # 3087
PIT_C = 889                 # scC: plane cols [3087, 3976)   -> chunk 7

_CACHE = {}


def _build():
    nc = bacc.Bacc("TRN2", target_bir_lowering=False, debug=False)
    z_in = nc.declare_dram_parameter("z", [SPC, C, KS, KS], f32, isOutput=False)
    x_in = nc.declare_dram_parameter("x", [SPC, C, MS, MS], f32, isOutput=False)
    w_in = nc.declare_dram_parameter("w", [C], f32, isOutput=False)
    out = nc.declare_dram_parameter("out", [SPC, 1, MO, MO], f32, isOutput=True)

    scA = [nc.dram_tensor(f"pl_scA{s}", [64, PIT_A], RT) for s in range(SPC)]
    scB = [nc.dram_tensor(f"pl_scB{s}", [64, PIT_B], RT) for s in range(SPC)]
    scC = [nc.dram_tensor(f"pl_scC{s}", [64, PIT_C], RT) for s in range(SPC)]

    # [SPC, 128, 2, F]: partition = channel-within-half, then half, pixels
    xsrc = x_in.rearrange("s (k c) h w -> s c k (h w)", c=128)

    from contextlib import ExitStack

    with tile.TileContext(nc) as tc:
        with ExitStack() as stack:
            pool = lambda name, bufs, **kw: stack.enter_context(
                tc.tile_pool(name=name, bufs=bufs, **kw)
            )
            xpools = [pool(f"xq{i}", 2) for i in range(4)]
            spools = [pool(f"sq{i}", 2) for i in range(4)]
            zpool = pool("zpool", 8)
            plApool = pool("plApool", 2)
            pl7pool = pool("pl7pool", 2)
            g1pool = pool("g1pool", 2)
            g2pool = pool("g2pool", 2)
            g3pool = pool("g3pool", 2)
            ohpool = pool("ohpool", 8)
            obpool = pool("obpool", 2)
            psum = pool("psum", 4, space="PSUM")
            psum2 = pool("psum2", 2, space="PSUM")

            # z/w first on the SWDGE ring so they land early
            wt = zpool.tile([128, 2], f32, name="wt")
            nc.gpsimd.dma_start(wt[:], w_in.rearrange("(k c) -> c k", c=128))
            zts = []
            for s in range(SPC):
                zt = zpool.tile([128, 2, KS * KS], f32, tag="zt", name=f"zt{s}")
                nc.gpsimd.dma_start(
                    zt[:], z_in[s].rearrange("(k c) p q -> c k (p q)", c=128)
                )
                zts.append(zt)
            w64 = zpool.tile([128, 2], f32, name="w64")
            nc.vector.tensor_scalar_mul(w64[:], wt[:], 1.0 / (KS * KS))

            # one-hot [64, 8] stationaries for the chunked tap reduction
            ohs = []
            for r in range(8):
                oh = ohpool.tile([64, 8], RT, name=f"oh{r}")
                nc.gpsimd.memset(oh[:], 0.0)
                nc.gpsimd.memset(oh[:, r : r + 1], 1.0)
                ohs.append(oh)

            # x pieces on the Sync HWDGE ring, samples interleaved
            xst = {}
            for pi, (b0, nbk) in enumerate(PIECES):
                for s in range(SPC):
                    lo = b0 * BLK
                    ln = min(nbk * BLK, F - lo)
                    t = xpools[pi].tile([128, 2, ln], f32, tag=f"xst{pi}",
                                        name=f"x{s}p{pi}")
                    nc.sync.dma_start(t[:], xsrc[s, :, :, lo : lo + ln])
                    xst[(s, pi)] = t

            # ---- sqrt pieces (ACT), z path interleaved after piece 0 ----
            sxt = {}

            def sqrt_piece(s, pi):
                b0, nbk = PIECES[pi]
                ln = min(nbk * BLK, F - b0 * BLK)
                t = spools[pi].tile([128, 2, ln], bf16, tag=f"sxp{pi}",
                                    name=f"sx{s}p{pi}")
                nc.scalar.activation(t[:], xst[(s, pi)][:], AF.Sqrt)
                sxt[(s, pi)] = t

            sqrt_piece(0, 0)
            sqrt_piece(1, 0)
            # szw[c, k, t] = w[c]/64 * sqrt(z[c, t])
            szws = []
            for s in range(SPC):
                zsq = zpool.tile([128, 2, KS * KS], f32, tag="zsq", name=f"zsq{s}")
                szw = zpool.tile([128, 2, KS * KS], bf16, tag="szw", name=f"szw{s}")
                for k in range(2):
                    nc.scalar.activation(zsq[:, k, :], zts[s][:, k, :], AF.Sqrt)
                    nc.vector.tensor_scalar_mul(
                        szw[:, k, :], zsq[:, k, :], w64[:, k : k + 1]
                    )
                szws.append(szw)
            for pi in range(1, 4):
                for s in range(SPC):
                    sqrt_piece(s, pi)

            # ---- stage 1: matmuls/casts/dumps, samples interleaved ----
            plAs, pl7s = {}, {}
            for s in range(SPC):
                plAs[s] = plApool.tile([64, 7 * BLK], RT, tag="plA",
                                       name=f"plA{s}")
                pl7s[s] = pl7pool.tile([64, 385], RT, tag="pl7", name=f"pl7{s}")

            def do_block(s, b):
                nb = NBLOCKS[b]
                pi = next(i for i, (b0, nbk) in enumerate(PIECES)
                          if b0 <= b < b0 + nbk)
                off = (b - PIECES[pi][0]) * BLK
                sx = sxt[(s, pi)]
                szw = szws[s]
                plA, pl7 = plAs[s], pl7s[s]
                ps = psum.tile([64, BLK], f32, tag="ps", name=f"ps{s}_{b}")
                for k in range(2):
                    nc.tensor.matmul(
                        ps[:, :nb],
                        szw[:, k, :],
                        sx[:, k, off : off + nb],
                        start=(k == 0),
                        stop=(k == 1),
                    )
                if b < 7:
                    nc.vector.tensor_copy(
                        plA[:, b * BLK : b * BLK + nb], ps[:, :nb]
                    )
                else:
                    nc.vector.tensor_copy(pl7[:, :nb], ps[:, :nb])
                # dumps as soon as their source region is complete
                if b == 5:
                    nc.sync.dma_start(scA[s][:, :PIT_A], plA[:, :PIT_A])
                elif b == 6:
                    nc.sync.dma_start(scB[s][:], plA[:, B_LO:3584])
                    nc.sync.dma_start(
                        scC[s][:, 0 : 3584 - C_LO], plA[:, C_LO:3584]
                    )
                elif b == 7:
                    nc.sync.dma_start(
                        scC[s][:, 3584 - C_LO : 3584 - C_LO + 385],
                        pl7[:, :385],
                    )

            for pi, (b0, nbk) in enumerate(PIECES):
                for s in range(SPC):
                    for b in range(b0, b0 + nbk):
                        do_block(s, b)

            # ---- stage 2 ----
            def gathers(s, eng):
                g1 = g1pool.tile([64, 5 * CW], RT, tag="g1", name=f"g1_{s}")
                eng.dma_start(g1[:], bass.AP(
                    scA[s][:].tensor, 0,
                    [[8 * PIT_A + MS, 8], [PIT_A + 1, 8], [CW, 5], [1, CW]],
                ))
                g2 = g2pool.tile([64, 2 * CW], RT, tag="g2", name=f"g2_{s}")
                eng.dma_start(g2[:], bass.AP(
                    scB[s][:].tensor, 0,
                    [[8 * PIT_B + MS, 8], [PIT_B + 1, 8], [CW, 2], [1, CW]],
                ))
                g3 = g3pool.tile([64, CW], RT, tag="g3", name=f"g3_{s}")
                eng.dma_start(g3[:], bass.AP(
                    scC[s][:].tensor, 0,
                    [[8 * PIT_C + MS, 8], [PIT_C + 1, 8], [1, CW]],
                ))
                return g1, g2, g3

            gts = [gathers(0, nc.scalar), gathers(1, nc.sync)]

            for s in range(SPC):
                g1, g2, g3 = gts[s]
                ps2 = psum2.tile([8, CW], f32, tag="ps2", name=f"ps2_{s}")
                for r in range(8):
                    src = g1 if r < 5 else (g2 if r < 7 else g3)
                    off = (r if r < 5 else (r - 5 if r < 7 else 0)) * CW
                    nc.tensor.matmul(
                        ps2[:, :CW],
                        ohs[r][:],
                        src[:, off : off + CW],
                        start=(r == 0),
                        stop=(r == 7),
                    )

                obuf = obpool.tile([8, 7 * MO], f32, tag="ob", name=f"ob{s}")
                psv = ps2[:].rearrange("p (i j) -> p i j", j=MS)[:, :, 0:MO]
                nc.vector.tensor_copy(
                    obuf[:].rearrange("p (i j) -> p i j", j=MO), psv
                )
                nc.scalar.dma_start(
                    out[s].rearrange("c (r i) j -> (c r) (i j)", r=8), obuf[:]
                )

    nc.compile()
    return nc


def _get_nc():
    if "nc" not in _CACHE:
        _CACHE["nc"] = _build()
    return _CACHE["nc"]


def _run(z, x, weights, **runkw):
    z = np.ascontiguousarray(np.asarray(z), dtype=np.float32)
    x = np.ascontiguousarray(np.asarray(x), dtype=np.float32)
    w = np.ascontiguousarray(np.asarray(weights), dtype=np.float32).reshape(C)
    in_maps = []
    for i in range(NCORES):
        lo, hi = i * SPC, (i + 1) * SPC
        in_maps.append({"z": z[lo:hi], "x": x[lo:hi], "w": w})
    nc = _get_nc()
    try:
        res = run_bass_kernel_spmd(
            nc, in_maps, core_ids=list(range(NCORES)), **runkw
        )
    except Exception:
        # transient device errors (e.g. NRT exec-unit unrecoverable) have
        # been observed to succeed on retry
        res = run_bass_kernel_spmd(
            nc, in_maps, core_ids=list(range(NCORES)), **runkw
        )
    full = np.concatenate([res.results[i]["out"] for i in range(NCORES)], axis=0)
    return full, res


def kernel(z, x, weights):
    full, _ = _run(z, x, weights)
    return full


# revision 11
# speedup vs baseline: 1.3103x; 1.0703x over previous
"""Bhattacharyya coefficient kernel for Trainium2 (8 NeuronCores, SPMD).

out[n,0,i,j] = (1/k^2) * sum_{c,p,q} w[c] * sqrt(x[n,c,i+p,j+q] * z[n,c,p,q])

Data-parallel over batch: 2 samples per core. Per sample:
  1. ACT: sx = sqrt(x) (bf16), szw = w/k^2 * sqrt(z) (bf16).
  2. TensorE: plane[t, y] = sum_c szw[c, t] * sx[c, y] for the 64 taps
     t = 8p+q and all 63*63 image pixels y (K=256 in two 128-chunks
     accumulated in PSUM, M=64 taps, N in blocks of <=512).
  3. Evict PSUM -> SBUF (bf16, DVE), dump plane to DRAM scratch.
  4. Gather back with per-tap shifted offsets (flat DRAM AP):
     A[t, 441*r + u] = plane[t, 441*r + u + 63*(t>>3) + (t&7)] for 8
     out-row chunks r (7 output rows each), turning the tap-sum into a
     pure partition reduction.  Three scratch tensors (chunks 0-4 /
     5-6 / 7) keep each gather waiting only on the dumps it covers.
  5. TensorE one-hot matmuls: ps[r, u] = sum_t A[t, 441*r + u] via a
     [64, 8] stationary whose only nonzero column is r, accumulating all
     8 chunks into one [8, 441] PSUM tile.  One DVE eviction compacts
     the valid 7x56 block per partition into obuf [8, 392]; one DMA
     ships the contiguous [56, 56] output.

x arrives in decreasing-size pieces (2048/1024/512/385 cols, both
channel halves per piece) with the two samples' pieces interleaved, so
both samples' serial tails (last piece -> sqrt -> matmul -> dump ->
gather -> reduce -> out) run concurrently right after the stream ends.
All gathers ride HWDGE rings (SWDGE completion is ~6us); z/w ride
SWDGE early where they don't contend with the x stream.
"""

import numpy as np

import concourse.bacc as bacc
import concourse.bass as bass
import concourse.mybir as mybir
from concourse import tile
from concourse.bass_utils import run_bass_kernel_spmd

N, C, KS, MS = 16, 256, 8, 63
MO = MS - KS + 1            # 56
F = MS * MS                 # 3969
NCORES = 8
SPC = N // NCORES           # samples per core
BLK = 512
AF = mybir.ActivationFunctionType
f32 = mybir.dt.float32
bf16 = mybir.dt.bfloat16

RT = mybir.dt.float8e4      # round-trip dtype for plane scratch

# x pieces (start block, n blocks): decreasing sizes
PIECES = [(0, 4), (4, 2), (6, 1), (7, 1)]
NBLOCKS = [(min(BLK, F - b * BLK)) for b in range(8)]   # 512 x7, 385

# stage-2: 8 out-row chunks of 7 rows; chunk r covers flat u in
# [441r, 441r+441), gathers plane cols [441r, 441r+889).
CW = 441
PIT_A = 6 * BLK             # scA: plane cols [0, 3072)      -> chunks 0-4
B_LO = 5 * CW               # 2205
PIT_B = 3584 - B_LO         # scB: plane cols [2205, 3584)   -> chunks 5-6
C_LO = 7 * CW               # 3087
PIT_C = 889                 # scC: plane cols [3087, 3976)   -> chunk 7

_CACHE = {}


def _build():
    nc = bacc.Bacc("TRN2", target_bir_lowering=False, debug=False)
    z_in = nc.declare_dram_parameter("zw", [SPC, 128, 2, 65], f32,
                                     isOutput=False)
    x_in = nc.declare_dram_parameter("x", [SPC, C, MS, MS], f32, isOutput=False)
    out = nc.declare_dram_parameter("out", [SPC, 1, MO, MO], f32, isOutput=True)

    scA = [nc.dram_tensor(f"pl_scA{s}", [64, PIT_A], RT) for s in range(SPC)]
    scB = [nc.dram_tensor(f"pl_scB{s}", [64, PIT_B], RT) for s in range(SPC)]
    scC = [nc.dram_tensor(f"pl_scC{s}", [64, PIT_C], RT) for s in range(SPC)]

    # [SPC, 128, 2, F]: partition = channel-within-half, then half, pixels
    xsrc = x_in.rearrange("s (k c) h w -> s c k (h w)", c=128)

    from contextlib import ExitStack

    with tile.TileContext(nc) as tc:
        with ExitStack() as stack:
            pool = lambda name, bufs, **kw: stack.enter_context(
                tc.tile_pool(name=name, bufs=bufs, **kw)
            )
            xpools = [pool(f"xq{i}", 2) for i in range(4)]
            spools = [pool(f"sq{i}", 2) for i in range(4)]
            zpool = pool("zpool", 8)
            plApool = pool("plApool", 2)
            pl7pool = pool("pl7pool", 2)
            g1pool = pool("g1pool", 2)
            g2pool = pool("g2pool", 2)
            g3pool = pool("g3pool", 2)
            ohpool = pool("ohpool", 8)
            obpool = pool("obpool", 2)
            psum = pool("psum", 4, space="PSUM")
            psum2 = pool("psum2", 2, space="PSUM")

            # zw (host-relayouted, w folded in col 64) first on sync:
            # 520B rows, lands early, no tiny-descriptor tax on the stream
            zts = []
            for s in range(SPC):
                zt = zpool.tile([128, 2, 65], f32, tag="zt", name=f"zt{s}")
                nc.sync.dma_start(zt[:], z_in[s])
                zts.append(zt)

            # one-hot [64, 8] stationaries for the chunked tap reduction
            ohs = []
            for r in range(8):
                oh = ohpool.tile([64, 8], RT, name=f"oh{r}")
                nc.gpsimd.memset(oh[:], 0.0)
                nc.gpsimd.memset(oh[:, r : r + 1], 1.0)
                ohs.append(oh)

            # x pieces on the Sync HWDGE ring, samples interleaved
            xst = {}
            for pi, (b0, nbk) in enumerate(PIECES):
                for s in range(SPC):
                    lo = b0 * BLK
                    ln = min(nbk * BLK, F - lo)
                    t = xpools[pi].tile([128, 2, ln], f32, tag=f"xst{pi}",
                                        name=f"x{s}p{pi}")
                    nc.sync.dma_start(t[:], xsrc[s, :, :, lo : lo + ln])
                    xst[(s, pi)] = t

            # ---- sqrt pieces (ACT), z path interleaved after piece 0 ----
            sxt = {}

            def sqrt_piece(s, pi):
                b0, nbk = PIECES[pi]
                ln = min(nbk * BLK, F - b0 * BLK)
                t = spools[pi].tile([128, 2, ln], bf16, tag=f"sxp{pi}",
                                    name=f"sx{s}p{pi}")
                nc.scalar.activation(t[:], xst[(s, pi)][:], AF.Sqrt)
                sxt[(s, pi)] = t

            # szw[c, k, t] = w[c]/64 * sqrt(z[c, t]); zw lands first so
            # these run before the x sqrts without stalling ACT
            szws = []
            for s in range(SPC):
                zsq = zpool.tile([128, 2, KS * KS], f32, tag="zsq", name=f"zsq{s}")
                szw = zpool.tile([128, 2, KS * KS], bf16, tag="szw", name=f"szw{s}")
                w64 = zpool.tile([128, 2], f32, tag="w64", name=f"w64_{s}")
                nc.vector.tensor_scalar_mul(
                    w64[:], zts[s][:, :, 64], 1.0 / (KS * KS)
                )
                for k in range(2):
                    nc.scalar.activation(
                        zsq[:, k, :], zts[s][:, k, 0:64], AF.Sqrt
                    )
                    nc.vector.tensor_scalar_mul(
                        szw[:, k, :], zsq[:, k, :], w64[:, k : k + 1]
                    )
                szws.append(szw)
            for pi in range(4):
                for s in range(SPC):
                    sqrt_piece(s, pi)

            # ---- stage 1: matmuls/casts/dumps, samples interleaved ----
            plAs, pl7s = {}, {}
            for s in range(SPC):
                plAs[s] = plApool.tile([64, 7 * BLK], RT, tag="plA",
                                       name=f"plA{s}")
                pl7s[s] = pl7pool.tile([64, 385], RT, tag="pl7", name=f"pl7{s}")

            def do_block(s, b):
                nb = NBLOCKS[b]
                pi = next(i for i, (b0, nbk) in enumerate(PIECES)
                          if b0 <= b < b0 + nbk)
                off = (b - PIECES[pi][0]) * BLK
                sx = sxt[(s, pi)]
                szw = szws[s]
                plA, pl7 = plAs[s], pl7s[s]
                ps = psum.tile([64, BLK], f32, tag="ps", name=f"ps{s}_{b}")
                for k in range(2):
                    nc.tensor.matmul(
                        ps[:, :nb],
                        szw[:, k, :],
                        sx[:, k, off : off + nb],
                        start=(k == 0),
                        stop=(k == 1),
                    )
                if b < 7:
                    nc.vector.tensor_copy(
                        plA[:, b * BLK : b * BLK + nb], ps[:, :nb]
                    )
                else:
                    nc.vector.tensor_copy(pl7[:, :nb], ps[:, :nb])
                # dumps as soon as their source region is complete
                if b == 5:
                    nc.sync.dma_start(scA[s][:, :PIT_A], plA[:, :PIT_A])
                elif b == 6:
                    nc.sync.dma_start(scB[s][:], plA[:, B_LO:3584])
                    nc.sync.dma_start(
                        scC[s][:, 0 : 3584 - C_LO], plA[:, C_LO:3584]
                    )
                elif b == 7:
                    nc.sync.dma_start(
                        scC[s][:, 3584 - C_LO : 3584 - C_LO + 385],
                        pl7[:, :385],
                    )

            for pi, (b0, nbk) in enumerate(PIECES):
                for s in range(SPC):
                    for b in range(b0, b0 + nbk):
                        do_block(s, b)

            # ---- stage 2 ----
            def gathers(s, eng):
                g1 = g1pool.tile([64, 5 * CW], RT, tag="g1", name=f"g1_{s}")
                eng.dma_start(g1[:], bass.AP(
                    scA[s][:].tensor, 0,
                    [[8 * PIT_A + MS, 8], [PIT_A + 1, 8], [CW, 5], [1, CW]],
                ))
                g2 = g2pool.tile([64, 2 * CW], RT, tag="g2", name=f"g2_{s}")
                eng.dma_start(g2[:], bass.AP(
                    scB[s][:].tensor, 0,
                    [[8 * PIT_B + MS, 8], [PIT_B + 1, 8], [CW, 2], [1, CW]],
                ))
                g3 = g3pool.tile([64, CW], RT, tag="g3", name=f"g3_{s}")
                eng.dma_start(g3[:], bass.AP(
                    scC[s][:].tensor, 0,
                    [[8 * PIT_C + MS, 8], [PIT_C + 1, 8], [1, CW]],
                ))
                return g1, g2, g3

            gts = [gathers(0, nc.scalar), gathers(1, nc.sync)]

            for s in range(SPC):
                g1, g2, g3 = gts[s]
                ps2 = psum2.tile([8, CW], f32, tag="ps2", name=f"ps2_{s}")
                for r in range(8):
                    src = g1 if r < 5 else (g2 if r < 7 else g3)
                    off = (r if r < 5 else (r - 5 if r < 7 else 0)) * CW
                    nc.tensor.matmul(
                        ps2[:, :CW],
                        ohs[r][:],
                        src[:, off : off + CW],
                        start=(r == 0),
                        stop=(r == 7),
                    )

                obuf = obpool.tile([8, 7 * MO], f32, tag="ob", name=f"ob{s}")
                psv = ps2[:].rearrange("p (i j) -> p i j", j=MS)[:, :, 0:MO]
                nc.vector.tensor_copy(
                    obuf[:].rearrange("p (i j) -> p i j", j=MO), psv
                )
                nc.scalar.dma_start(
                    out[s].rearrange("c (r i) j -> (c r) (i j)", r=8), obuf[:]
                )

    nc.compile()
    return nc


def _get_nc():
    if "nc" not in _CACHE:
        _CACHE["nc"] = _build()
    return _CACHE["nc"]


def _run(z, x, weights, **runkw):
    z = np.asarray(z, dtype=np.float32)
    x = np.ascontiguousarray(np.asarray(x), dtype=np.float32)
    w = np.asarray(weights, dtype=np.float32).reshape(C)
    # host relayout: zw[n, c, k, 0:64] = z[n, 128k+c, p, q]; col 64 = w
    zw = np.empty((N, 128, 2, 65), dtype=np.float32)
    zw[:, :, :, 0:64] = z.reshape(N, 2, 128, KS * KS).transpose(0, 2, 1, 3)
    zw[:, :, :, 64] = w.reshape(2, 128).T[None]
    zw = np.ascontiguousarray(zw)
    in_maps = []
    for i in range(NCORES):
        lo, hi = i * SPC, (i + 1) * SPC
        in_maps.append({"zw": zw[lo:hi], "x": x[lo:hi]})
    nc = _get_nc()
    try:
        res = run_bass_kernel_spmd(
            nc, in_maps, core_ids=list(range(NCORES)), **runkw
        )
    except Exception:
        # transient device errors (e.g. NRT exec-unit unrecoverable) have
        # been observed to succeed on retry
        res = run_bass_kernel_spmd(
            nc, in_maps, core_ids=list(range(NCORES)), **runkw
        )
    full = np.concatenate([res.results[i]["out"] for i in range(NCORES)], axis=0)
    return full, res


def kernel(z, x, weights):
    full, _ = _run(z, x, weights)
    return full


# revision 12
# speedup vs baseline: 1.4200x; 1.0837x over previous
"""Bhattacharyya coefficient kernel for Trainium2 (8 NeuronCores, SPMD).

out[n,0,i,j] = (1/k^2) * sum_{c,p,q} w[c] * sqrt(x[n,c,i+p,j+q] * z[n,c,p,q])

Data-parallel over batch: 2 samples per core. Per sample:
  1. ACT: sx = sqrt(x) (fp8e4m3), szw = w/k^2 * sqrt(z) (fp8e4m3).
  2. TensorE fp8 DoubleRow: plane[t, y] = sum_c szw[c, t] * sx[c, y]
     for the 64 taps t = 8p+q and all 63*63 pixels y -- K=256 contracted
     in ONE matmul per <=512-column block (two 128-channel tiles per
     pass), M=64 taps.
  3. Evict PSUM -> SBUF (fp8, DVE), dump plane to DRAM scratch.
  4. Gather back with per-tap shifted offsets (flat DRAM AP):
     A[t, 441*r + u] = plane[t, 441*r + u + 63*(t>>3) + (t&7)] for 8
     out-row chunks r (7 output rows each), turning the tap-sum into a
     pure partition reduction.  Three scratch tensors (chunks 0-4 /
     5-6 / 7) keep each gather waiting only on the dumps it covers.
  5. TensorE one-hot matmuls: ps[r, u] = sum_t A[t, 441*r + u] via a
     [64, 8] fp8 stationary whose only nonzero column is r, accumulating
     all 8 chunks into one [8, 441] PSUM tile.  One DVE eviction
     compacts the valid 7x56 block per partition into obuf [8, 392];
     one DMA ships the contiguous [56, 56] output.

x arrives in decreasing-size pieces (1024x3/512/256/129 cols, both
channel halves per piece) with the two samples' pieces interleaved, so
both samples' serial tails (last piece -> sqrt -> matmul -> dump ->
gather -> reduce -> out) run concurrently right after the stream ends.
z is host-relayouted to [128, 2, 65] (channel-major, w packed in col
64) so it loads early with 520B descriptors.  All gathers ride HWDGE
rings (SWDGE completion is ~6us slower).
"""

import numpy as np

import concourse.bacc as bacc
import concourse.bass as bass
import concourse.mybir as mybir
from concourse import tile
from concourse.bass_utils import run_bass_kernel_spmd

N, C, KS, MS = 16, 256, 8, 63
MO = MS - KS + 1            # 56
F = MS * MS                 # 3969
NCORES = 8
SPC = N // NCORES           # samples per core
BLK = 512
AF = mybir.ActivationFunctionType
f32 = mybir.dt.float32
fp8 = mybir.dt.float8e4
DR = mybir.MatmulPerfMode.DoubleRow

RT = fp8                    # round-trip dtype for plane scratch

# x pieces (col_start, n_cols): decreasing sizes; last two split block 7
PIECES = [(0, 1024), (1024, 1024), (2048, 1024), (3072, 512),
          (3584, 256), (3840, 129)]
NBLOCKS = [(min(BLK, F - b * BLK)) for b in range(8)]   # 512 x7, 385

# stage-2: 8 out-row chunks of 7 rows; chunk r covers flat u in
# [441r, 441r+441), gathers plane cols [441r, 441r+889).
CW = 441
PIT_A = 6 * BLK             # scA: plane cols [0, 3072)      -> chunks 0-4
B_LO = 5 * CW               # 2205
PIT_B = 3584 - B_LO         # scB: plane cols [2205, 3584)   -> chunks 5-6
C_LO = 7 * CW               # 3087
PIT_C = 889                 # scC: plane cols [3087, 3976)   -> chunk 7

_CACHE = {}


def _build():
    nc = bacc.Bacc("TRN2", target_bir_lowering=False, debug=False)
    z_in = nc.declare_dram_parameter("zw", [SPC, 128, 2, 65], f32,
                                     isOutput=False)
    x_in = nc.declare_dram_parameter("x", [SPC, C, MS, MS], f32, isOutput=False)
    out = nc.declare_dram_parameter("out", [SPC, 1, MO, MO], f32, isOutput=True)

    scA = [nc.dram_tensor(f"pl_scA{s}", [64, PIT_A], RT) for s in range(SPC)]
    scB = [nc.dram_tensor(f"pl_scB{s}", [64, PIT_B], RT) for s in range(SPC)]
    scC = [nc.dram_tensor(f"pl_scC{s}", [64, PIT_C], RT) for s in range(SPC)]

    # [SPC, 128, 2, F]: partition = channel-within-half, then half, pixels
    xsrc = x_in.rearrange("s (k c) h w -> s c k (h w)", c=128)

    from contextlib import ExitStack

    with tile.TileContext(nc) as tc:
        with ExitStack() as stack:
            pool = lambda name, bufs, **kw: stack.enter_context(
                tc.tile_pool(name=name, bufs=bufs, **kw)
            )
            xpools = [pool(f"xq{i}", 2) for i in range(len(PIECES))]
            spools = [pool(f"sq{i}", 2) for i in range(len(PIECES))]
            zpool = pool("zpool", 8)
            plApool = pool("plApool", 2)
            pl7pool = pool("pl7pool", 2)
            g1pool = pool("g1pool", 2)
            g2pool = pool("g2pool", 2)
            g3pool = pool("g3pool", 2)
            ohpool = pool("ohpool", 8)
            obpool = pool("obpool", 2)
            psum = pool("psum", 4, space="PSUM")
            psum2 = pool("psum2", 2, space="PSUM")

            # zw (host-relayouted, w folded in col 64) first on sync:
            # 520B rows, lands early, no tiny-descriptor tax on the stream
            zts = []
            for s in range(SPC):
                zt = zpool.tile([128, 2, 65], f32, tag="zt", name=f"zt{s}")
                nc.sync.dma_start(zt[:], z_in[s])
                zts.append(zt)

            # one-hot [64, 8] stationaries for the chunked tap reduction
            ohs = []
            for r in range(8):
                oh = ohpool.tile([64, 8], RT, name=f"oh{r}")
                nc.gpsimd.memset(oh[:], 0.0)
                nc.gpsimd.memset(oh[:, r : r + 1], 1.0)
                ohs.append(oh)

            # x pieces on the Sync HWDGE ring, samples interleaved
            xst = {}
            for pi, (lo, ln) in enumerate(PIECES):
                for s in range(SPC):
                    t = xpools[pi].tile([128, 2, ln], f32, tag=f"xst{pi}",
                                        name=f"x{s}p{pi}")
                    nc.sync.dma_start(t[:], xsrc[s, :, :, lo : lo + ln])
                    xst[(s, pi)] = t

            # szw[c, k, t] = w[c]/64 * sqrt(z[c, t]); zw lands first so
            # these run before the x sqrts without stalling ACT
            szws = []
            for s in range(SPC):
                zsq = zpool.tile([128, 2, KS * KS], f32, tag="zsq",
                                 name=f"zsq{s}")
                szw = zpool.tile([128, 2, KS * KS], fp8, tag="szw",
                                 name=f"szw{s}")
                w64 = zpool.tile([128, 2], f32, tag="w64", name=f"w64_{s}")
                nc.vector.tensor_scalar_mul(
                    w64[:], zts[s][:, :, 64], 1.0 / (KS * KS)
                )
                for k in range(2):
                    nc.scalar.activation(
                        zsq[:, k, :], zts[s][:, k, 0:64], AF.Sqrt
                    )
                    nc.vector.tensor_scalar_mul(
                        szw[:, k, :], zsq[:, k, :], w64[:, k : k + 1]
                    )
                szws.append(szw)

            # ---- stage 1, piecewise, samples interleaved ----
            sxt = {}
            plAs, pl7s, ps7s = {}, {}, {}
            for s in range(SPC):
                plAs[s] = plApool.tile([64, 7 * BLK], RT, tag="plA",
                                       name=f"plA{s}")
                pl7s[s] = pl7pool.tile([64, 385], RT, tag="pl7", name=f"pl7{s}")

            def do_piece(s, pi):
                lo, ln = PIECES[pi]
                t = spools[pi].tile([128, 2, ln], fp8, tag=f"sxp{pi}",
                                    name=f"sx{s}p{pi}")
                nc.scalar.activation(t[:], xst[(s, pi)][:], AF.Sqrt)
                sxt[(s, pi)] = t
                szw = szws[s]
                plA, pl7 = plAs[s], pl7s[s]
                if pi < 4:
                    # whole 512-blocks lo/BLK .. (lo+ln)/BLK
                    for b in range(lo // BLK, (lo + ln) // BLK):
                        off = b * BLK - lo
                        ps = psum.tile([64, BLK], f32, tag="ps",
                                       name=f"ps{s}_{b}")
                        nc.tensor.matmul(
                            ps[:, :BLK], szw[:],
                            t[:, :, off : off + BLK],
                            start=True, stop=True, perf_mode=DR,
                        )
                        nc.vector.tensor_copy(
                            plA[:, b * BLK : (b + 1) * BLK], ps[:, :BLK]
                        )
                        if b == 5:
                            nc.sync.dma_start(scA[s][:, :PIT_A],
                                              plA[:, :PIT_A])
                        elif b == 6:
                            nc.sync.dma_start(scB[s][:], plA[:, B_LO:3584])
                            nc.sync.dma_start(
                                scC[s][:, 0 : 3584 - C_LO], plA[:, C_LO:3584]
                            )
                else:
                    # block 7 sub-ranges (cols lo-3584 .. +ln within block 7)
                    o7 = lo - 7 * BLK
                    if pi == 4:
                        ps7s[s] = psum.tile([64, 385], f32, tag="ps",
                                            name=f"ps{s}_7")
                    ps = ps7s[s]
                    nc.tensor.matmul(
                        ps[:, o7 : o7 + ln], szw[:], t[:],
                        start=True, stop=True, perf_mode=DR,
                    )
                    if pi == 5:
                        nc.vector.tensor_copy(pl7[:, :385], ps[:, :385])
                        nc.sync.dma_start(
                            scC[s][:, 3584 - C_LO : 3584 - C_LO + 385],
                            pl7[:, :385],
                        )

            for pi in range(len(PIECES)):
                for s in range(SPC):
                    do_piece(s, pi)

            # ---- stage 2 ----
            def gathers(s, eng):
                g1 = g1pool.tile([64, 5 * CW], RT, tag="g1", name=f"g1_{s}")
                eng.dma_start(g1[:], bass.AP(
                    scA[s][:].tensor, 0,
                    [[8 * PIT_A + MS, 8], [PIT_A + 1, 8], [CW, 5], [1, CW]],
                ))
                g2 = g2pool.tile([64, 2 * CW], RT, tag="g2", name=f"g2_{s}")
                eng.dma_start(g2[:], bass.AP(
                    scB[s][:].tensor, 0,
                    [[8 * PIT_B + MS, 8], [PIT_B + 1, 8], [CW, 2], [1, CW]],
                ))
                g3 = g3pool.tile([64, CW], RT, tag="g3", name=f"g3_{s}")
                eng.dma_start(g3[:], bass.AP(
                    scC[s][:].tensor, 0,
                    [[8 * PIT_C + MS, 8], [PIT_C + 1, 8], [1, CW]],
                ))
                return g1, g2, g3

            gts = [gathers(0, nc.scalar), gathers(1, nc.sync)]

            for s in range(SPC):
                g1, g2, g3 = gts[s]
                ps2 = psum2.tile([8, CW], f32, tag="ps2", name=f"ps2_{s}")
                for r in range(8):
                    src = g1 if r < 5 else (g2 if r < 7 else g3)
                    off = (r if r < 5 else (r - 5 if r < 7 else 0)) * CW
                    nc.tensor.matmul(
                        ps2[:, :CW],
                        ohs[r][:],
                        src[:, off : off + CW],
                        start=(r == 0),
                        stop=(r == 7),
                    )

                obuf = obpool.tile([8, 7 * MO], f32, tag="ob", name=f"ob{s}")
                psv = ps2[:].rearrange("p (i j) -> p i j", j=MS)[:, :, 0:MO]
                nc.vector.tensor_copy(
                    obuf[:].rearrange("p (i j) -> p i j", j=MO), psv
                )
                nc.scalar.dma_start(
                    out[s].rearrange("c (r i) j -> (c r) (i j)", r=8), obuf[:]
                )

    nc.compile()
    return nc


def _get_nc():
    if "nc" not in _CACHE:
        _CACHE["nc"] = _build()
    return _CACHE["nc"]


def _run(z, x, weights, **runkw):
    z = np.asarray(z, dtype=np.float32)
    x = np.ascontiguousarray(np.asarray(x), dtype=np.float32)
    w = np.asarray(weights, dtype=np.float32).reshape(C)
    # host relayout: zw[n, c, k, 0:64] = z[n, 128k+c, p, q]; col 64 = w
    zw = np.empty((N, 128, 2, 65), dtype=np.float32)
    zw[:, :, :, 0:64] = z.reshape(N, 2, 128, KS * KS).transpose(0, 2, 1, 3)
    zw[:, :, :, 64] = w.reshape(2, 128).T[None]
    zw = np.ascontiguousarray(zw)
    in_maps = []
    for i in range(NCORES):
        lo, hi = i * SPC, (i + 1) * SPC
        in_maps.append({"zw": zw[lo:hi], "x": x[lo:hi]})
    nc = _get_nc()
    try:
        res = run_bass_kernel_spmd(
            nc, in_maps, core_ids=list(range(NCORES)), **runkw
        )
    except Exception:
        # transient device errors (e.g. NRT exec-unit unrecoverable) have
        # been observed to succeed on retry
        res = run_bass_kernel_spmd(
            nc, in_maps, core_ids=list(range(NCORES)), **runkw
        )
    full = np.concatenate([res.results[i]["out"] for i in range(NCORES)], axis=0)
    return full, res


def kernel(z, x, weights):
    full, _ = _run(z, x, weights)
    return full
